# revision 1
# baseline (speedup 1.0000x reference)
"""AR LSTM decoder kernel for Trainium2, data-parallel over 8 NeuronCores.

Problem: per-step LSTM cell + FC(5) + log_softmax + argmax + class-embedding
feedback, B=1024, T=1024, IN=HIDDEN=64. Batch is sharded 128 rows/core; the
T=1024 recurrence runs on-device per core. State is kept in [feature, batch]
layout; the only transpose is the [B,5] argmax one-hot -> [5,B] on the PE.

Optimizations vs the fp32 baseline (~5.1us/step -> ~4.4us/step cost model):
- all gate/fc/transpose matmuls and the recurrent state in bf16 (fp32 PE
  matmul is 4 cycles/row, bf16 is 1; bf16 also unlocks DVE 2x perf modes);
  PSUM accumulation and the logits/argmax compare stay fp32
- ONE sigmoid over all 4 gates: g-gate weights are pre-scaled by 2 on host
  and tanh(g) = 2*sigmoid(2g) - 1 is recovered with a single tensor_scalar
  (2 ALU ops), cutting ACT from 3 gate activations/step to 1
- the next step's gate-bank matmuls are emitted as soon as their inputs
  exist: x-parts after the DMA, h-parts right after h is written (they
  overlap the argmax tail), leaving only the two K=5 one-hot matmuls
  between the feedback copy and the next sigmoid
- fc logits / one-hot transpose get their own PSUM banks (no accumulation
  ordering helpers against the gate bank)

Walrus constraints honored: no InstISA (custom DVE) ops; TensorTensor
in0/in1 share a start partition (out may differ); matmul rhs base
partition must be 0/32/64 (hence the separate one-hot tile).

log_softmax is applied on host (argmax(logits) == argmax(log_softmax(logits)),
so the device only needs biased logits for both output and feedback).
"""
import os
import sys

sys.path.insert(0, "/opt/trn_rl_repo")

import numpy as np
import ml_dtypes

import concourse.bass as bass
import concourse.tile as tile
from concourse.tile import add_dep_helper
from concourse import mybir
from concourse.bass_utils import run_bass_kernel_spmd

P = 128          # batch rows per core
IN = 64
H = 64
NCLS = 5
NCORES = 8
XB = 64          # time steps per x-block DMA
S_OUT = 128      # time steps per output accumulation chunk
KL = 101         # late-contraction rows: h(64) + ones(1) + pad(31) + ohT(5)

F32 = mybir.dt.float32
BF16 = mybir.dt.bfloat16
BF_NP = ml_dtypes.bfloat16
# cell-state / sigmoid-output dtype: bf16 halves DVE elementwise time
# (2x perf mode); measured rel err 6.5e-3 vs the 2e-2 gate
CELL_BF16 = os.environ.get("CELL_BF16", "1") == "1"
CDT = BF16 if CELL_BF16 else F32


def _split_excess_waits(nc, cap=1):
    """This walrus build accepts at most one sync-wait per instruction; move
    excess waits onto preceding same-engine NOPs (equivalent ordering)."""
    n_new = 0
    for f in nc.m.functions:
        for bb in f.blocks:
            new_list = None
            for idx, inst in enumerate(bb.instructions):
                si = inst.sync_info
                waits = list(si.on_wait) if si and si.on_wait else []
                if len(waits) > cap:
                    if new_list is None:
                        new_list = list(bb.instructions[:idx])
                    extra, keep = waits[:-cap], waits[-cap:]
                    for w in extra:
                        n_new += 1
                        new_list.append(mybir.InstNoOp(
                            name=f"waitsplit-{n_new}-{inst.name}",
                            sync_info=mybir.SyncInfo(on_wait=[w], on_update=[]),
                            bass_nofuse=True,
                            engine=inst.engine,
                        ))
                    inst.sync_info = mybir.SyncInfo(
                        on_wait=keep, on_update=list(si.on_update or []))
                    new_list.append(inst)
                elif new_list is not None:
                    new_list.append(inst)
            if new_list is not None:
                bb.instructions = new_list
    return n_new


def _build(t_steps):
    """Latency-bound recurrence: per-step time is one dependency-chain
    traversal: late matmuls -> sigma(all gates) -> 2t-1 fixup -> cell ->
    tanh(c) -> h -> fc -> max -> one-hot -> transpose -> feedback copy."""
    nc = bass.Bass("TRN2", target_bir_lowering=False)

    xT = nc.dram_tensor("xT", [t_steps, IN, P], BF16, kind="ExternalInput")[:]
    y = nc.dram_tensor("y", [P, t_steps * NCLS], F32, kind="ExternalOutput")[:]
    w_if_x = nc.dram_tensor("w_if_x", [IN, 128], BF16, kind="ExternalInput")[:]
    w_if_h = nc.dram_tensor("w_if_h", [H + 1, 128], BF16, kind="ExternalInput")[:]
    w_if_o = nc.dram_tensor("w_if_o", [NCLS, 128], BF16, kind="ExternalInput")[:]
    w_go_x = nc.dram_tensor("w_go_x", [IN, 128], BF16, kind="ExternalInput")[:]
    w_go_h = nc.dram_tensor("w_go_h", [H + 1, 128], BF16, kind="ExternalInput")[:]
    w_go_o = nc.dram_tensor("w_go_o", [NCLS, 128], BF16, kind="ExternalInput")[:]
    wfcb = nc.dram_tensor("wfcb", [H + 1, NCLS], BF16, kind="ExternalInput")[:]
    ident = nc.dram_tensor("ident", [P, P], BF16, kind="ExternalInput")[:]

    sig = mybir.ActivationFunctionType.Sigmoid
    tanh = mybir.ActivationFunctionType.Tanh

    with tile.TileContext(nc) as tc:
        with (
            tc.tile_pool(name="const", bufs=1) as const,
            tc.tile_pool(name="state", bufs=1) as state,
            tc.tile_pool(name="xblk", bufs=3) as xblk,
            tc.tile_pool(name="work", bufs=3) as work,
            tc.tile_pool(name="acc", bufs=2) as accp,
            tc.tile_pool(name="psG", bufs=2, space="PSUM") as psG,
            tc.tile_pool(name="psL", bufs=2, space="PSUM") as psL,
            tc.tile_pool(name="psO", bufs=2, space="PSUM") as psO,
        ):
            # constants
            c_wifx = const.tile([IN, 128], BF16, tag="wifx")
            c_wifh = const.tile([H + 1, 128], BF16, tag="wifh")
            c_wifo = const.tile([NCLS, 128], BF16, tag="wifo")
            c_wgox = const.tile([IN, 128], BF16, tag="wgox")
            c_wgoh = const.tile([H + 1, 128], BF16, tag="wgoh")
            c_wgoo = const.tile([NCLS, 128], BF16, tag="wgoo")
            c_wfcb = const.tile([H + 1, NCLS], BF16, tag="wfcb")
            c_id = const.tile([P, P], BF16, tag="ident")
            for dst, src in ((c_wifx, w_if_x), (c_wifh, w_if_h),
                             (c_wifo, w_if_o), (c_wgox, w_go_x),
                             (c_wgoh, w_go_h), (c_wgoo, w_go_o),
                             (c_wfcb, wfcb), (c_id, ident)):
                nc.sync.dma_start(out=dst[:], in_=src)

            # persistent state: rows 0:64 h (bf16), row 64 ones (bias);
            # the one-hot feedback lives in its own tile (matmul rhs base
            # partition must be 0/32/64)
            scat = state.tile([H + 1, P], BF16, tag="scat")
            ohsb = state.tile([NCLS, P], BF16, tag="ohsb")
            # c lives at base partition 64 so the f-half of sigma outputs
            # (partitions 64:128) can multiply it with matching bases
            c_t = state.tile([128, P], CDT, tag="c")
            nc.vector.memset(scat[:], 0.0)
            nc.vector.memset(scat[64:65, :], 1.0)
            nc.vector.memset(ohsb[:], 0.0)
            nc.vector.memset(c_t[:], 0.0)

            acc = None
            xb = None
            bankG = None
            mm1 = None

            def emit_xmms(t):
                """allocate the gate bank for step t and start its two
                x-contribution matmuls (they only need the x DMA)"""
                nonlocal xb, bankG, mm1
                sb, so_in_blk = divmod(t, XB)
                if so_in_blk == 0:
                    nsteps = min(XB, t_steps - sb * XB)
                    xb = xblk.tile([IN, XB, P], BF16, tag="xb")
                    nc.sync.dma_start(
                        out=xb[:, :nsteps, :],
                        in_=xT[sb * XB: sb * XB + nsteps].rearrange("t f b -> f t b"),
                    )
                x_t = xb[:, so_in_blk, :]
                bankG = psG.tile([128, 2 * P], F32, tag="bankG")
                mm1 = nc.tensor.matmul(bankG[:, 0:P], c_wifx[:], x_t,
                                       start=True, stop=False,
                                       skip_group_check=True)
                mm_gx = nc.tensor.matmul(bankG[:, P:2 * P], c_wgox[:], x_t,
                                         start=False, stop=False,
                                         skip_group_check=True)
                add_dep_helper(mm_gx.ins, mm1.ins, sync=False,
                               reason="bank start order")

            def emit_h_mms():
                """h-dependent gate matmuls: fire as soon as h is written,
                while the argmax tail is still running"""
                for m in (
                    nc.tensor.matmul(bankG[:, 0:P], c_wifh[:],
                                     scat[0:H + 1, :], start=False,
                                     stop=False, skip_group_check=True),
                    nc.tensor.matmul(bankG[:, P:2 * P], c_wgoh[:],
                                     scat[0:H + 1, :], start=False,
                                     stop=False, skip_group_check=True),
                ):
                    add_dep_helper(m.ins, mm1.ins, sync=False,
                                   reason="bank start order")

            def emit_oh_mms():
                """one-hot feedback matmuls: tiny K=5 weight load, the only
                matmul work left between the feedback copy and sigma"""
                for m in (
                    nc.tensor.matmul(bankG[:, 0:P], c_wifo[:],
                                     ohsb[:], start=False,
                                     stop=True, skip_group_check=True),
                    nc.tensor.matmul(bankG[:, P:2 * P], c_wgoo[:],
                                     ohsb[:], start=False,
                                     stop=True, skip_group_check=True),
                ):
                    add_dep_helper(m.ins, mm1.ins, sync=False,
                                   reason="bank start order")

            emit_xmms(0)
            emit_h_mms()
            emit_oh_mms()
            for t in range(t_steps):
                myG = bankG
                if t % S_OUT == 0:
                    acc = accp.tile([P, S_OUT * NCLS], F32, tag="acc")
                s5 = (t % S_OUT) * NCLS

                # ---- one sigmoid over all four gates (g pre-scaled by 2):
                # sg: [sig_i ; sig_f] cols 0:P, [tau_g ; sig_o] cols P:2P
                sg = work.tile([128, 2 * P], CDT, tag="sg")
                nc.scalar.activation(sg[:], myG[:, 0:2 * P], sig)

                # ---- cell: c' = sig_f*c + sig_i*(2*tau_g - 1)
                tg2 = work.tile([128, P], CDT, tag="tg2")
                nc.vector.tensor_scalar(tg2[0:64, :], sg[0:64, P:2 * P], 2.0,
                                        1.0, op0=mybir.AluOpType.mult,
                                        op1=mybir.AluOpType.subtract)
                m2t = work.tile([128, P], CDT, tag="m2")
                nc.vector.tensor_mul(m2t[64:128, :], sg[0:64, 0:P], tg2[0:64, :])
                m1t = work.tile([128, P], CDT, tag="m1")
                nc.vector.tensor_mul(m1t[64:128, :], sg[64:128, 0:P],
                                     c_t[64:128, :])
                nc.vector.tensor_add(c_t[64:128, :], m1t[64:128, :],
                                     m2t[64:128, :])
                tc_ = work.tile([128, P], CDT, tag="tc")
                nc.scalar.activation(tc_[64:128, :], c_t[64:128, :], tanh)
                nc.vector.tensor_mul(scat[0:64, :], sg[64:128, P:2 * P],
                                     tc_[64:128, :])

                # next step's bank: x matmuls (need only the x DMA) and
                # h matmuls (need only the h just written) start here and
                # overlap with the whole argmax tail below
                if t + 1 < t_steps:
                    emit_xmms(t + 1)
                    emit_h_mms()

                # ---- fc logits (bias folded via the ones row), own bank
                ps_log = psL.tile([P, NCLS], F32, tag="pslog")
                nc.tensor.matmul(ps_log[:], scat[0:H + 1, :], c_wfcb[:],
                                 start=True, stop=True, skip_group_check=True)

                # ---- argmax -> one-hot -> transpose -> feedback rows
                mx = work.tile([128, 1], F32, tag="mx")
                nc.vector.reduce_max(mx[:], ps_log[:], axis=mybir.AxisListType.X)
                onehot = work.tile([P, NCLS], BF16, tag="oh")
                nc.vector.tensor_scalar(onehot[:], ps_log[:], mx[:], None,
                                        op0=mybir.AluOpType.is_ge)
                ps_oh = psO.tile([NCLS, P], BF16, tag="psoh")
                nc.tensor.transpose(ps_oh[:], onehot[:], c_id[:])
                nc.vector.tensor_copy(ohsb[:], ps_oh[:])
                if t + 1 < t_steps:
                    emit_oh_mms()

                # off-chain: logits into the output accumulator on ACT (its
                # queue is near-idle; keeps DVE SEQ free for chain ops)
                nc.scalar.copy(acc[:, s5:s5 + NCLS], ps_log[:])

                if (t + 1) % S_OUT == 0 or t == t_steps - 1:
                    t0 = (t // S_OUT) * S_OUT
                    ncols = (t - t0 + 1) * NCLS
                    nc.sync.dma_start(
                        out=y[:, t0 * NCLS: t0 * NCLS + ncols],
                        in_=acc[:, :ncols],
                    )

    _split_excess_waits(nc, cap=1)
    return nc


_BUILT = {}


def _get_nc(t_steps):
    if t_steps not in _BUILT:
        _BUILT[t_steps] = _build(t_steps)
    return _BUILT[t_steps]


def _prep_maps(x, W_ih, W_hh, b_ih, b_hh, W_fc, b_fc, emb, t_steps):
    x = np.asarray(x, np.float32)
    W_ih = np.asarray(W_ih, np.float32)
    W_hh = np.asarray(W_hh, np.float32)
    b = (np.asarray(b_ih, np.float32) + np.asarray(b_hh, np.float32))
    W_fc = np.asarray(W_fc, np.float32)
    b_fc = np.asarray(b_fc, np.float32)
    emb = np.asarray(emb, np.float32)

    # g-gate pre-scaling: tanh(g) = 2*sigmoid(2g) - 1, fold the 2 into all
    # g-gate weight/bias columns (g occupies cols 0:64 of the go-half)
    gscale = np.ones((128,), np.float32)
    gscale[0:64] = 2.0

    def hpart(wr, br):       # [W_hh_gate.T ; bias] -> [H+1, 128]
        return np.vstack([wr.T, br[None, :]])

    com = {
        "w_if_x": np.ascontiguousarray(W_ih[0:128, 0:64].T).astype(BF_NP),
        "w_if_h": np.ascontiguousarray(
            hpart(W_hh[0:128], b[0:128])).astype(BF_NP),
        "w_if_o": np.ascontiguousarray(
            emb @ W_ih[0:128, 64:128].T).astype(BF_NP),
        "w_go_x": np.ascontiguousarray(
            W_ih[128:256, 0:64].T * gscale[None, :]).astype(BF_NP),
        "w_go_h": np.ascontiguousarray(
            hpart(W_hh[128:256], b[128:256]) * gscale[None, :]).astype(BF_NP),
        "w_go_o": np.ascontiguousarray(
            (emb @ W_ih[128:256, 64:128].T) * gscale[None, :]).astype(BF_NP),
        "wfcb": np.ascontiguousarray(
            np.vstack([W_fc.T, b_fc[None, :]])).astype(BF_NP),
        "ident": np.eye(P, dtype=np.float32).astype(BF_NP),
    }

    # one pass over x: [B,T,IN] -> per-core [T, IN, P] shards, all contiguous
    xs = np.ascontiguousarray(
        x.reshape(NCORES, P, t_steps, IN).transpose(0, 2, 3, 1)).astype(BF_NP)
    maps = []
    for cidx in range(NCORES):
        m = dict(com)
        m["xT"] = xs[cidx]
        maps.append(m)
    return maps


def kernel(x, x_lengths=None, edge_list=None, W_ih=None, W_hh=None,
           b_ih=None, b_hh=None, W_fc=None, b_fc=None, emb=None, **_):
    x = np.asarray(x, np.float32)
    B, t_steps, _ = x.shape
    assert B == P * NCORES
    nc = _get_nc(t_steps)
    maps = _prep_maps(x, W_ih, W_hh, b_ih, b_hh, W_fc, b_fc, emb, t_steps)
    res = run_bass_kernel_spmd(nc, maps, core_ids=list(range(NCORES)))
    shards = [res.results[i]["y"].reshape(P, t_steps, NCLS) for i in range(NCORES)]
    logits = np.concatenate(shards, axis=0)
    m = logits.max(-1, keepdims=True)
    logp = logits - m - np.log(np.exp(logits - m).sum(-1, keepdims=True))
    return logp.astype(np.float32)

